# revision 44
# baseline (speedup 1.0000x reference)
"""Longformer sliding-window + global attention layer on 8 Trainium2 NeuronCores.

Sharding: sequence-parallel over the 4096 tokens (512 per core, all 12 heads).
Each core recomputes the k/v halo (256 tokens each side) and the 64 global
k/v tokens locally from zero-padded hsT input, so the program is uniform SPMD.
The global-query rows (first 64 tokens attend to everything) are computed as
flash-style partial sums over each core's 512 tokens and combined with an
on-device AllReduce (hidden under the banded phase); every core finalizes the
identical 64 global rows.

Layout strategy (all matmuls bf16, accumulation fp32 PSUM):
  - hsT [hidden, tokens] feeds projections in both orientations. The 64
    global tokens appear twice at the tail (cols 1024:1088 and 1088:1152) so
    the natural-v projection materializes global v rows in both partition
    halves; the second copy feeds the odd head of each pair as a PE row-tile
    T8 operand so the two heads' K=64 global-key PV matmuls pack.
  - Banded attention: per extended-window key tile jx (8 tiles of 128 keys),
    scores are computed transposed ([keys, queries]) over exactly the query
    span the band touches (128/256/384/512 wide), 17% less than 256-chunking.
    The two heads of a pair use disjoint PE row groups (tile_position (0,0)
    vs (64,0)) with score matmuls interleaved per-jx. Key tiles pair up as
    (3,4),(2,7),(5,0),(1,6) sharing one 2-bank PSUM tile and one exp per
    head covering both tiles; pair offsets land on {0,384,512} because a
    matmul output must not cross the 512-col PSUM bank boundary. Each pair
    needs one contiguous 0/1 mask multiply per head. The head-pair loop is
    software-pipelined: scores(hp) are emitted before PV(hp-1) so dense
    score bursts alternate with PV streams while exp/mask of the previous
    pair run on the scalar/vector engines.
  - PV accumulates into a [66, 512] PSUM tile via natural-v tiles carrying an
    appended ones-column, so the softmax denominator falls out of the same
    accumulation. The raw [66, 512] block (values + denominator row) is
    written out per head; the host divides and transposes, eliminating all
    on-device PE transposes and the normalize chain.
  - Global-row heads compute scores directly transposed via four 128-key
    strip matmuls per head (row-tile packed across the pair), two heads
    sharing one PSUM tile and one exp, then PV with exp strips stationary.
"""
import numpy as np
import ml_dtypes

import concourse.bacc as bacc
import concourse.mybir as mybir
import concourse.tile as tile
from concourse.bass_utils import run_bass_kernel_spmd

F32 = mybir.dt.float32
BF16 = mybir.dt.bfloat16
Exp = mybir.ActivationFunctionType.Exp

S, H, NH, HD = 4096, 768, 12, 64
C = 256               # chunk / one-sided window
G = 64                # global tokens
NCORE = 8
TPC = S // NCORE      # 512 tokens per core
EXT = TPC + 2 * C     # 1024 ext window
COLS = EXT + 2 * G    # 1152 = ext | glob | glob-dup
KC = H // 128         # 6 hidden chunks
VW = 66               # per-head v block: 64 v | ones | pad
OUTR = NH * VW        # 792 output rows ([66,512] block per head)
SCALE = 1.0 / 8.0     # 1/sqrt(HD)

# per key tile jx: exact query span the band touches (T0, width)
JXW = {0: 128, 1: 256, 2: 384, 3: 512, 4: 512, 5: 384, 6: 256, 7: 128}
JXT0 = {0: 0, 1: 0, 2: 0, 3: 0, 4: 0, 5: 128, 6: 256, 7: 384}
# pairs share one [128, 1024] PSUM tile; (3,4) first so PV start=True covers
# the full accumulator. Pair offsets must land on {0, 384, 512}: a matmul
# output may not cross the 512-col PSUM bank boundary.
JX_PAIRS = [(3, 4), (2, 7), (5, 0), (1, 6)]
# mask ops: (pair_index, ex-tile col range, packed col range) - one
# contiguous multiply per (head, pair)
MASK_OPS = [
    (0, (384, 640), (0, 256)),
    (1, (0, 512), (256, 768)),
    (2, (0, 512), (768, 1280)),
    (3, (0, 512), (1280, 1792)),
]
MCOLS = 1792

_PROG_CACHE = {}


def _build_program(with_bias: bool):
    nc = bacc.Bacc("TRN2", target_bir_lowering=False, debug=False,
                   num_devices=NCORE)
    d_hsT = nc.declare_dram_parameter("hsT", [H, COLS], BF16, isOutput=False)
    d_w = {
        n: nc.declare_dram_parameter(n, [H, H], BF16, isOutput=False)
        for n in ("wq", "wk", "wv", "wkg", "wvg", "wqg")
    }
    d_masks = nc.declare_dram_parameter("masks", [128, MCOLS], BF16,
                                        isOutput=False)
    if with_bias:
        d_brow = nc.declare_dram_parameter("biasrow", [7, COLS], BF16,
                                           isOutput=False)
    d_out = nc.declare_dram_parameter("out", [OUTR, TPC], BF16, isOutput=True)
    d_outg = nc.declare_dram_parameter("outg", [G, NH * VW], F32,
                                       isOutput=True)

    with tile.TileContext(nc) as tc:
        with (
            tc.tile_pool(name="const", bufs=1) as const,
            tc.tile_pool(name="wfull", bufs=2) as wfull,
            tc.tile_pool(name="work", bufs=2) as work,
            tc.tile_pool(name="w2", bufs=3) as w2,
            tc.tile_pool(name="late", bufs=1) as late,
            tc.tile_pool(name="dram", bufs=2, space="DRAM") as dram,
            tc.tile_pool(name="psQ", bufs=3, space="PSUM") as psQ,
            tc.tile_pool(name="psO", bufs=2, space="PSUM") as psO,
        ):
            QS = [nc.sync, nc.scalar, nc.gpsimd]

            # ---- resident loads. Startup order matters: wq + hsb mid cols
            # land first (q projection starts ~5us in), then wk so the k
            # projection never stalls. Whole [128,768] rows per DMA - finer
            # strips are DMA-latency-bound, not bandwidth-bound.
            hsb = late.tile([128, KC, COLS], BF16, tag="ph")
            qi = 0

            def load_w(name, t=None):
                nonlocal qi
                if t is None:
                    t = wfull.tile([128, KC, H], BF16, tag="wfull")
                for kc in range(KC):
                    QS[qi % 3].dma_start(
                        out=t[:, kc, :],
                        in_=d_w[name][128 * kc:128 * (kc + 1), :])
                    qi += 1
                return t

            wq_t = wfull.tile([128, KC, H], BF16, tag="wq")
            for kc in range(KC):
                QS[qi % 3].dma_start(
                    out=wq_t[:, kc, :],
                    in_=d_w["wq"][128 * kc:128 * (kc + 1), :])
                qi += 1
                QS[qi % 3].dma_start(
                    out=hsb[:, kc, 256:768],
                    in_=d_hsT[128 * kc:128 * (kc + 1), 256:768])
                qi += 1
            wk_t = load_w("wk")
            for kc in range(KC):
                QS[qi % 3].dma_start(
                    out=hsb[:, kc, 0:256],
                    in_=d_hsT[128 * kc:128 * (kc + 1), 0:256])
                qi += 1
                QS[qi % 3].dma_start(
                    out=hsb[:, kc, 768:COLS],
                    in_=d_hsT[128 * kc:128 * (kc + 1), 768:COLS])
                qi += 1

            if with_bias:
                bsb = const.tile([7, COLS], BF16)
                nc.gpsimd.dma_start(out=bsb, in_=d_brow[:])

            KCOLS = EXT + G                           # kT covers one glob copy
            kT = const.tile([128, KC, KCOLS], BF16)   # [o, t] all heads
            qT = const.tile([128, KC, TPC], BF16)
            vE = const.tile([128, 9, NH * VW], BF16)  # natural v + ones cols
            kgT = const.tile([128, KC, TPC], BF16)
            vgN = const.tile([128, 4, NH * VW], BF16)
            qgT = const.tile([128, KC, G], BF16)
            msb = const.tile([128, MCOLS], BF16)
            nc.gpsimd.dma_start(out=msb, in_=d_masks[:])
            # ones/pad columns of the natural-v blocks (cols 64,65 of each
            # 66-block); value cols are overwritten by the projections
            nc.gpsimd.memset(
                vE.rearrange("p a (h x) -> p a h x", x=VW)[:, :, :, 64:66], 1.0)
            nc.gpsimd.memset(
                vgN.rearrange("p a (h x) -> p a h x", x=VW)[:, :, :, 64:66], 1.0)

            def proj_T(dst, wsl, segs, bias_idx, dst_off, copy_eng=None):
                # dst[o, t] = W.T @ hsT cols; wsl(kc, oc) -> [128, 128] strip
                ce = copy_eng or nc.vector
                cp = ce.tensor_copy if ce is not nc.scalar else ce.copy
                for oc in range(KC):
                    for c0, cn in segs:
                        ps = psQ.tile([128, 512], F32, tag="psQ")
                        for kc in range(KC):
                            nc.tensor.matmul(
                                out=ps[:, 0:cn],
                                lhsT=wsl(kc, oc),
                                rhs=hsb[:, kc, c0:c0 + cn],
                                start=(kc == 0),
                                stop=(kc == KC - 1 and not with_bias),
                            )
                        if with_bias:
                            nc.tensor.matmul(
                                out=ps[:, 0:cn],
                                lhsT=bsb[1 + bias_idx:2 + bias_idx,
                                         oc * 128:(oc + 1) * 128],
                                rhs=bsb[0:1, 0:cn],
                                start=False, stop=True,
                            )
                        cp(out=dst[:, oc, c0 - dst_off:c0 - dst_off + cn],
                           in_=ps[:, 0:cn])

            def proj_nat(dst, wsb, tts, bias_idx):
                # dst[t, head-block] with 66-stride head blocks
                for ti, tt in enumerate(tts):
                    tok0 = tt * 128
                    for o0, on in ((0, 512), (512, 256)):
                        ps = psQ.tile([128, 512], F32, tag="psQ")
                        for kc in range(KC):
                            nc.tensor.matmul(
                                out=ps[:, 0:on],
                                lhsT=hsb[:, kc, tok0:tok0 + 128],
                                rhs=wsb[:, kc, o0:o0 + on],
                                start=(kc == 0),
                                stop=(kc == KC - 1 and not with_bias),
                            )
                        if with_bias:
                            nc.tensor.matmul(
                                out=ps[:, 0:on],
                                lhsT=bsb[0:1, 0:128],
                                rhs=bsb[1 + bias_idx:2 + bias_idx, o0:o0 + on],
                                start=False, stop=True,
                            )
                        nc.vector.tensor_copy(
                            out=dst[:, ti, :].rearrange(
                                "p (h x) -> p h x", x=VW)[:, o0 // 64:(o0 + on) // 64, 0:64],
                            in_=ps[:, 0:on].rearrange("p (h x) -> p h x", x=64))

            # ---- main projections first (banded inputs ready earliest) ----
            # q projection kc-outer: all 6 oc accumulators live in PSUM at
            # once (3 x 2-bank tiles), so the PE starts when the first
            # (wq kc-strip, hsb kc-strip) DMA pair lands and streams while
            # the remaining strips arrive.
            qps = [psQ.tile([128, 1024], F32, tag="psQ", name=f"qps{i}")
                   for i in range(3)]
            for kc in range(KC):
                for oc in range(KC):
                    nc.tensor.matmul(
                        out=qps[oc // 2][:, 512 * (oc % 2):512 * (oc % 2) + 512],
                        lhsT=wq_t[:, kc, oc * 128:(oc + 1) * 128],
                        rhs=hsb[:, kc, C:C + 512],
                        start=(kc == 0),
                        stop=(kc == KC - 1 and not with_bias),
                    )
            for oc in range(KC):
                if with_bias:
                    nc.tensor.matmul(
                        out=qps[oc // 2][:, 512 * (oc % 2):512 * (oc % 2) + 512],
                        lhsT=bsb[1:2, oc * 128:(oc + 1) * 128],
                        rhs=bsb[0:1, 0:512], start=False, stop=True)
                nc.vector.tensor_copy(
                    out=qT[:, oc, :],
                    in_=qps[oc // 2][:, 512 * (oc % 2):512 * (oc % 2) + 512])
            proj_T(kT,
                   lambda kc, oc: wk_t[:, kc, oc * 128:(oc + 1) * 128],
                   ((256, 512), (0, 256), (768, 320)), 1, 0)
            w = load_w("wv")
            proj_nat(vE, w, (0, 1, 2, 3, 4, 5, 6, 7, 8), 2)

            # ---- global-row projections + partials (overlap banded) ----
            w = load_w("wkg")
            proj_T(kgT, lambda kc, oc, _w=w: _w[:, kc, oc * 128:(oc + 1) * 128],
                   ((C, 512),), 3, C)
            w = load_w("wvg")
            proj_nat(vgN, w, (2, 3, 4, 5), 4)
            w = load_w("wqg")
            # qgT [o, t] directly (N=64 matmuls; no PE transposes needed)
            proj_T(qgT, lambda kc, oc, _w=w: _w[:, kc, oc * 128:(oc + 1) * 128],
                   ((EXT, G),), 5, EXT)

            partial = dram.tile([G, NH * VW], F32)
            reduced = dram.tile([G, NH * VW], F32)

            def glob_pair(g0):
                # two global-row heads (g0 even, g0+1) share one PSUM tile and
                # one exp; scores land transposed ([keys, 64]) so PV needs no
                # transposes. Score matmuls alternate PE row tiles and pack.
                g1 = g0 + 1
                pse = psQ.tile([128, 1024], F32, tag="psQ", name=f"pse{g0}")
                for kt in range(4):
                    for gi, gh in enumerate((g0, g1)):
                        dd = 64 * (gh % 2)
                        nc.tensor.matmul(
                            out=pse[:, 512 * gi + kt * G:512 * gi + (kt + 1) * G],
                            lhsT=kgT[dd:dd + 64, gh // 2,
                                     128 * kt:128 * (kt + 1)],
                            rhs=qgT[dd:dd + 64, gh // 2, :],
                            start=True, stop=True)
                exgT = work.tile([128, 512], BF16, tag="exgT", name=f"exgT{g0}")
                for gi in range(2):
                    nc.scalar.activation(out=exgT[:, 256 * gi:256 * (gi + 1)],
                                         in_=pse[:, 512 * gi:512 * gi + 256],
                                         func=Exp, scale=SCALE)
                for gi, gh in enumerate((g0, g1)):
                    ppv = psO.tile([VW, TPC], F32, tag="psO", name=f"ppv{gh}")
                    for kt in range(4):
                        nc.tensor.matmul(
                            out=ppv[0:G, 0:VW],
                            lhsT=exgT[:, 256 * gi + kt * G:256 * gi + (kt + 1) * G],
                            rhs=vgN[:, kt, VW * gh:VW * (gh + 1)],
                            start=(kt == 0), stop=(kt == 3))
                    part = w2.tile([G, VW], F32, tag="part", name=f"part{gh}")
                    nc.vector.tensor_copy(out=part, in_=ppv[0:G, 0:VW])
                    nc.sync.dma_start(out=partial[:, gh * VW:(gh + 1) * VW],
                                      in_=part)

            # ---- banded + global-column attention (the bulk) ----
            # heads processed in pairs on disjoint PE row tiles with score
            # matmuls interleaved per key tile so the PE packs them; the
            # global-row pairs dovetail into the first banded pairs so the
            # AllReduce fires early and hides under the remaining banded work
            def emit_scores(hp):
                h0, h1 = 2 * hp, 2 * hp + 1
                pc = hp
                # -- scores for both heads, row tiles interleaved per jx --
                exs = {h0: [], h1: []}
                for pi, (pa, pb) in enumerate(JX_PAIRS):
                    wa, wb = JXW[pa], JXW[pb]
                    pss = {}
                    for hh in (h0, h1):
                        pss[hh] = psQ.tile([128, 1024], F32, tag="psQ",
                                           name=f"pss{hh}_{pa}")
                        ex = work.tile([128, 1024], BF16, tag="ex", bufs=18)
                        exs[hh].append(ex)
                    for jx, off in ((pa, 0), (pb, wa)):
                        t0 = JXT0[jx]
                        tn = t0 + JXW[jx]
                        for hh in (h0, h1):
                            dd = 64 * (hh % 2)
                            nc.tensor.matmul(
                                out=pss[hh][:, off:off + tn - t0],
                                lhsT=kT[dd:dd + 64, pc,
                                        128 * jx:128 * (jx + 1)],
                                rhs=qT[dd:dd + 64, pc, t0:tn],
                                start=True, stop=True)
                    # exp then mask per pair so the masks drain on vector
                    # while later pairs' scores still stream on the PE
                    _, (c0, c1), (k0, k1) = MASK_OPS[pi]
                    for hh in (h0, h1):
                        nc.scalar.activation(out=exs[hh][-1][:, 0:wa + wb],
                                             in_=pss[hh][:, 0:wa + wb],
                                             func=Exp, scale=SCALE)
                        nc.vector.tensor_mul(
                            exs[hh][-1][:, c0:c1], exs[hh][-1][:, c0:c1],
                            msb[:, k0:k1])
                # global-key columns for both heads; h1 lands in PSUM
                # partitions 64:128 of bank 2 (quadrant (64,64)) so its exp
                # stays lane-aligned and the K=64 PV tails pack as T0/T8
                pssg = psQ.tile([128, 1024], F32, tag="psQ", name=f"pssg{h0}")
                exg = work.tile([128, TPC], BF16, tag="exg", name=f"exg{h0}",
                                bufs=3)
                nc.tensor.matmul(
                    out=pssg[0:G, 0:TPC],
                    lhsT=kT[0:64, pc, EXT:EXT + G],
                    rhs=qT[0:64, pc, :],
                    start=True, stop=True)
                nc.tensor.matmul(
                    out=pssg[64:64 + G, 512:512 + TPC],
                    lhsT=kT[64:128, pc, EXT:EXT + G],
                    rhs=qT[64:128, pc, :],
                    start=True, stop=True)
                nc.scalar.activation(out=exg[0:G, :], in_=pssg[0:G, 0:TPC],
                                     func=Exp, scale=SCALE)
                nc.scalar.activation(out=exg[64:64 + G, :],
                                     in_=pssg[64:64 + G, 512:512 + TPC],
                                     func=Exp, scale=SCALE)
                return exs, exg

            def emit_pv(hp, exs, exg):
                h0, h1 = 2 * hp, 2 * hp + 1
                # -- PV per head; K=64 global tails adjacent so they pack --
                pso = {}
                for hh in (h0, h1):
                    pso[hh] = psO.tile([VW, TPC], F32, tag="psO",
                                       name=f"pso{hh}")
                    first_pv = True
                    for (pa, pb), ex in zip(JX_PAIRS, exs[hh]):
                        for jx, off in ((pa, 0), (pb, JXW[pa])):
                            t0 = JXT0[jx]
                            nc.tensor.matmul(
                                out=pso[hh][:, t0:t0 + JXW[jx]],
                                lhsT=vE[:, jx, VW * hh:VW * (hh + 1)],
                                rhs=ex[:, off:off + JXW[jx]],
                                start=first_pv, stop=False)
                            first_pv = False
                for hh, dd in ((h0, 0), (h1, 64)):
                    nc.tensor.matmul(
                        out=pso[hh],
                        lhsT=vE[dd:dd + G, 8, VW * hh:VW * (hh + 1)],
                        rhs=exg[dd:dd + G, :], start=False, stop=True)
                # raw [66,512] out block (values + denominator row); host
                # divides and transposes
                for i, hh in enumerate((h0, h1)):
                    ot = w2.tile([VW, TPC], BF16, tag="ot")
                    # split the two PSUM-evacuation casts across engines so
                    # the psO banks recycle ~0.6us earlier
                    if i == 0:
                        nc.vector.tensor_copy(out=ot, in_=pso[hh])
                    else:
                        nc.scalar.copy(out=ot, in_=pso[hh])
                    QS[(hp * 2 + i) % 2].dma_start(
                        out=d_out[VW * hh:VW * (hh + 1), :], in_=ot)

            # software pipeline: scores(hp) run while PV(hp-1) streams, so
            # the 64-row score bursts stay contiguous and pack on row tiles
            prev = None
            for hp in range(NH // 2):
                if hp < 2:
                    glob_pair(6 * hp)
                    glob_pair(6 * hp + 2)
                    glob_pair(6 * hp + 4)
                if hp == 2:
                    nc.gpsimd.collective_compute(
                        "AllReduce", mybir.AluOpType.add,
                        replica_groups=[list(range(NCORE))],
                        ins=[partial.opt()], outs=[reduced.opt()])
                    red = late.tile([G, NH * VW], F32, tag="red")
                    nc.gpsimd.dma_start(out=red, in_=reduced)
                cur = emit_scores(hp)
                if prev is not None:
                    emit_pv(hp - 1, *prev)
                prev = cur
            emit_pv(NH // 2 - 1, *prev)

            # ---- ship the AllReduced global-row partials raw; the host
            # divides by the denominator columns (same as the banded blocks).
            # Only this one tail DMA depends on the collective, so a slow
            # peer core cannot stall any engine queue mid-kernel.
            nc.sync.dma_start(out=d_outg[:], in_=red)

    nc.compile()
    return nc


def _host_inputs(hs, weights, biases):
    """Build the 8 per-core input maps from full inputs."""
    BF = ml_dtypes.bfloat16
    hsT = np.ascontiguousarray(hs.T).astype(BF)    # [H, S]
    weights_bf = [w.astype(BF) for w in weights]

    with_bias = any(np.any(b) for b in biases)
    if with_bias:
        brow = np.zeros((7, COLS), BF)
        brow[0, :] = 1.0
        for i, b in enumerate(biases):
            brow[1 + i, :H] = b.astype(BF)
    pp = np.arange(128)[:, None]                    # key pos within jx tile
    in_maps = []
    for core in range(NCORE):
        hst = np.zeros((H, COLS), BF)
        lo = TPC * core - C
        hi = TPC * core + TPC + C
        clo, chi = max(lo, 0), min(hi, S)
        hst[:, clo - lo:chi - lo] = hsT[:, clo:chi]
        hst[:, EXT:EXT + G] = hsT[:, :G]
        hst[:, EXT + G:] = hsT[:, :G]               # duplicate global tokens
        # masks packed per pair: for packed col -> (jx, query q); valid iff
        # in-band and the absolute key is a real non-global token
        mk = np.ones((128, MCOLS), BF)
        for pi, (c0, c1), (k0, k1) in MASK_OPS:
            pa, pb = JX_PAIRS[pi]
            wa = JXW[pa]
            for ci in range(c0, c1):
                jx = pa if ci < wa else pb
                off = ci if ci < wa else ci - wa
                q = JXT0[jx] + off
                k_loc = 128 * jx - 256 + pp[:, 0]
                ka = TPC * core + k_loc
                valid = (np.abs(k_loc - q) <= C) & (ka >= G) & (ka < S)
                mk[:, k0 + (ci - c0)] = valid
        im = {
            "hsT": hst,
            "wq": weights_bf[0], "wk": weights_bf[1], "wv": weights_bf[2],
            "wkg": weights_bf[3], "wvg": weights_bf[4], "wqg": weights_bf[5],
            "masks": mk,
        }
        if with_bias:
            im["biasrow"] = brow
        in_maps.append(im)
    return in_maps, with_bias


def kernel(hidden_states, Wq, bq, Wk, bk, Wv, bv, Wqg, bqg, Wkg, bkg,
           Wvg, bvg):
    hs = np.asarray(hidden_states, np.float32).reshape(S, H)
    weights = [np.ascontiguousarray(np.asarray(w, np.float32))
               for w in (Wq, Wk, Wv, Wkg, Wvg, Wqg)]
    biases = [np.asarray(b, np.float32)
              for b in (bq, bk, bv, bkg, bvg, bqg)]
    in_maps, with_bias = _host_inputs(hs, weights, biases)

    if with_bias not in _PROG_CACHE:
        _PROG_CACHE[with_bias] = _build_program(with_bias)
    nc = _PROG_CACHE[with_bias]

    res = run_bass_kernel_spmd(nc, in_maps, list(range(NCORE)))

    out = np.empty((S, H), np.float32)
    for core in range(NCORE):
        blk = np.asarray(res.results[core]["out"], np.float32)
        blk = blk.reshape(NH, VW, TPC)
        norm = blk[:, :HD, :] / blk[:, HD:HD + 1, :]     # [NH, HD, TPC]
        out[TPC * core:TPC * (core + 1)] = (
            norm.transpose(2, 0, 1).reshape(TPC, H))
    og = np.asarray(res.results[0]["outg"], np.float32).reshape(G, NH, VW)
    out[:G] = (og[:, :, :HD] / og[:, :, HD:HD + 1]).reshape(G, H)
    return out.reshape(1, S, H)


# revision 45
# speedup vs baseline: 1.0219x; 1.0219x over previous
"""Longformer sliding-window + global attention layer on 8 Trainium2 NeuronCores.

Sharding: sequence-parallel over the 4096 tokens (512 per core, all 12 heads).
Each core recomputes the k/v halo (256 tokens each side) and the 64 global
k/v tokens locally from zero-padded hsT input, so the program is uniform SPMD.
The global-query rows (first 64 tokens attend to everything) are computed as
flash-style partial sums over each core's 512 tokens and combined with an
on-device AllReduce (hidden under the banded phase); every core finalizes the
identical 64 global rows.

Layout strategy (all matmuls bf16, accumulation fp32 PSUM):
  - hsT [hidden, tokens] feeds projections in both orientations. The 64
    global tokens appear twice at the tail (cols 1024:1088 and 1088:1152) so
    the natural-v projection materializes global v rows in both partition
    halves; the second copy feeds the odd head of each pair as a PE row-tile
    T8 operand so the two heads' K=64 global-key PV matmuls pack.
  - Banded attention: per extended-window key tile jx (8 tiles of 128 keys),
    scores are computed transposed ([keys, queries]) over exactly the query
    span the band touches (128/256/384/512 wide), 17% less than 256-chunking.
    The two heads of a pair use disjoint PE row groups (tile_position (0,0)
    vs (64,0)) with score matmuls interleaved per-jx. Key tiles pair up as
    (3,4),(2,7),(5,0),(1,6) sharing one 2-bank PSUM tile and one exp per
    head covering both tiles; pair offsets land on {0,384,512} because a
    matmul output must not cross the 512-col PSUM bank boundary. Each pair
    needs one contiguous 0/1 mask multiply per head. The head-pair loop is
    software-pipelined: scores(hp) are emitted before PV(hp-1) so dense
    score bursts alternate with PV streams while exp/mask of the previous
    pair run on the scalar/vector engines.
  - PV accumulates into a [66, 512] PSUM tile via natural-v tiles carrying an
    appended ones-column, so the softmax denominator falls out of the same
    accumulation. The raw [66, 512] block (values + denominator row) is
    written out per head; the host divides and transposes, eliminating all
    on-device PE transposes and the normalize chain.
  - Global-row heads compute scores directly transposed via four 128-key
    strip matmuls per head (row-tile packed across the pair), two heads
    sharing one PSUM tile and one exp, then PV with exp strips stationary.
"""
import numpy as np
import ml_dtypes

import concourse.bacc as bacc
import concourse.mybir as mybir
import concourse.tile as tile
from concourse.bass_utils import run_bass_kernel_spmd

F32 = mybir.dt.float32
BF16 = mybir.dt.bfloat16
Exp = mybir.ActivationFunctionType.Exp

S, H, NH, HD = 4096, 768, 12, 64
C = 256               # chunk / one-sided window
G = 64                # global tokens
NCORE = 8
TPC = S // NCORE      # 512 tokens per core
EXT = TPC + 2 * C     # 1024 ext window
COLS = EXT + 2 * G    # 1152 = ext | glob | glob-dup
KC = H // 128         # 6 hidden chunks
VW = 66               # per-head v block: 64 v | ones | pad
OUTR = NH * VW        # 792 output rows ([66,512] block per head)
SCALE = 1.0 / 8.0     # 1/sqrt(HD)

# per key tile jx: exact query span the band touches (T0, width)
JXW = {0: 128, 1: 256, 2: 384, 3: 512, 4: 512, 5: 384, 6: 256, 7: 128}
JXT0 = {0: 0, 1: 0, 2: 0, 3: 0, 4: 0, 5: 128, 6: 256, 7: 384}
# pairs share one [128, 1024] PSUM tile; (3,4) first so PV start=True covers
# the full accumulator. Pair offsets must land on {0, 384, 512}: a matmul
# output may not cross the 512-col PSUM bank boundary.
JX_PAIRS = [(3, 4), (2, 7), (5, 0), (1, 6)]
# mask ops: (pair_index, ex-tile col range, packed col range) - one
# contiguous multiply per (head, pair)
MASK_OPS = [
    (0, (384, 640), (0, 256)),
    (1, (0, 512), (256, 768)),
    (2, (0, 512), (768, 1280)),
    (3, (0, 512), (1280, 1792)),
]
MCOLS = 1792

_PROG_CACHE = {}


def _build_program(with_bias: bool):
    nc = bacc.Bacc("TRN2", target_bir_lowering=False, debug=False,
                   num_devices=NCORE)
    d_hsT = nc.declare_dram_parameter("hsT", [H, COLS], BF16, isOutput=False)
    d_w = {
        n: nc.declare_dram_parameter(n, [H, H], BF16, isOutput=False)
        for n in ("wq", "wk", "wv", "wkg", "wvg", "wqg")
    }
    d_masks = nc.declare_dram_parameter("masks", [128, MCOLS], BF16,
                                        isOutput=False)
    if with_bias:
        d_brow = nc.declare_dram_parameter("biasrow", [7, COLS], BF16,
                                           isOutput=False)
    d_out = nc.declare_dram_parameter("out", [OUTR, TPC], BF16, isOutput=True)
    d_outg = nc.declare_dram_parameter("outg", [G, NH * VW], F32,
                                       isOutput=True)

    with tile.TileContext(nc) as tc:
        with (
            tc.tile_pool(name="const", bufs=1) as const,
            tc.tile_pool(name="wfull", bufs=2) as wfull,
            tc.tile_pool(name="work", bufs=2) as work,
            tc.tile_pool(name="w2", bufs=3) as w2,
            tc.tile_pool(name="late", bufs=1) as late,
            tc.tile_pool(name="dram", bufs=2, space="DRAM") as dram,
            tc.tile_pool(name="psQ", bufs=3, space="PSUM") as psQ,
            tc.tile_pool(name="psO", bufs=2, space="PSUM") as psO,
        ):
            QS = [nc.sync, nc.scalar, nc.gpsimd]

            # ---- resident loads. Startup order matters: wq + hsb mid cols
            # land first (q projection starts ~5us in), then wk so the k
            # projection never stalls. Whole [128,768] rows per DMA - finer
            # strips are DMA-latency-bound, not bandwidth-bound.
            hsb = late.tile([128, KC, COLS], BF16, tag="ph")
            qi = 0

            def load_w(name, t=None):
                nonlocal qi
                if t is None:
                    t = wfull.tile([128, KC, H], BF16, tag="wfull")
                for kc in range(KC):
                    QS[qi % 3].dma_start(
                        out=t[:, kc, :],
                        in_=d_w[name][128 * kc:128 * (kc + 1), :])
                    qi += 1
                return t

            wq_t = wfull.tile([128, KC, H], BF16, tag="wq")
            for kc in range(KC):
                QS[qi % 3].dma_start(
                    out=wq_t[:, kc, :],
                    in_=d_w["wq"][128 * kc:128 * (kc + 1), :])
                qi += 1
                QS[qi % 3].dma_start(
                    out=hsb[:, kc, 256:768],
                    in_=d_hsT[128 * kc:128 * (kc + 1), 256:768])
                qi += 1
            wk_t = load_w("wk")
            for kc in range(KC):
                QS[qi % 3].dma_start(
                    out=hsb[:, kc, 0:256],
                    in_=d_hsT[128 * kc:128 * (kc + 1), 0:256])
                qi += 1
                QS[qi % 3].dma_start(
                    out=hsb[:, kc, 768:COLS],
                    in_=d_hsT[128 * kc:128 * (kc + 1), 768:COLS])
                qi += 1

            if with_bias:
                bsb = const.tile([7, COLS], BF16)
                nc.gpsimd.dma_start(out=bsb, in_=d_brow[:])

            KCOLS = EXT + G                           # kT covers one glob copy
            kT = const.tile([128, KC, KCOLS], BF16)   # [o, t] all heads
            qT = const.tile([128, KC, TPC], BF16)
            vE = const.tile([128, 9, NH * VW], BF16)  # natural v + ones cols
            kgT = const.tile([128, KC, TPC], BF16)
            vgN = const.tile([128, 4, NH * VW], BF16)
            qgT = const.tile([128, KC, G], BF16)
            msb = const.tile([128, MCOLS], BF16)
            nc.gpsimd.dma_start(out=msb, in_=d_masks[:])
            # ones/pad columns of the natural-v blocks (cols 64,65 of each
            # 66-block); value cols are overwritten by the projections
            nc.gpsimd.memset(
                vE.rearrange("p a (h x) -> p a h x", x=VW)[:, :, :, 64:66], 1.0)
            nc.gpsimd.memset(
                vgN.rearrange("p a (h x) -> p a h x", x=VW)[:, :, :, 64:66], 1.0)

            def proj_T(dst, wsl, segs, bias_idx, dst_off, copy_eng=None):
                # dst[o, t] = W.T @ hsT cols; wsl(kc, oc) -> [128, 128] strip
                ce = copy_eng or nc.vector
                cp = ce.tensor_copy if ce is not nc.scalar else ce.copy
                for oc in range(KC):
                    for c0, cn in segs:
                        ps = psQ.tile([128, 512], F32, tag="psQ")
                        for kc in range(KC):
                            nc.tensor.matmul(
                                out=ps[:, 0:cn],
                                lhsT=wsl(kc, oc),
                                rhs=hsb[:, kc, c0:c0 + cn],
                                start=(kc == 0),
                                stop=(kc == KC - 1 and not with_bias),
                            )
                        if with_bias:
                            nc.tensor.matmul(
                                out=ps[:, 0:cn],
                                lhsT=bsb[1 + bias_idx:2 + bias_idx,
                                         oc * 128:(oc + 1) * 128],
                                rhs=bsb[0:1, 0:cn],
                                start=False, stop=True,
                            )
                        cp(out=dst[:, oc, c0 - dst_off:c0 - dst_off + cn],
                           in_=ps[:, 0:cn])

            def proj_nat(dst, wsb, tts, bias_idx):
                # dst[t, head-block] with 66-stride head blocks
                for ti, tt in enumerate(tts):
                    tok0 = tt * 128
                    for o0, on in ((0, 512), (512, 256)):
                        ps = psQ.tile([128, 512], F32, tag="psQ")
                        for kc in range(KC):
                            nc.tensor.matmul(
                                out=ps[:, 0:on],
                                lhsT=hsb[:, kc, tok0:tok0 + 128],
                                rhs=wsb[:, kc, o0:o0 + on],
                                start=(kc == 0),
                                stop=(kc == KC - 1 and not with_bias),
                            )
                        if with_bias:
                            nc.tensor.matmul(
                                out=ps[:, 0:on],
                                lhsT=bsb[0:1, 0:128],
                                rhs=bsb[1 + bias_idx:2 + bias_idx, o0:o0 + on],
                                start=False, stop=True,
                            )
                        nc.vector.tensor_copy(
                            out=dst[:, ti, :].rearrange(
                                "p (h x) -> p h x", x=VW)[:, o0 // 64:(o0 + on) // 64, 0:64],
                            in_=ps[:, 0:on].rearrange("p (h x) -> p h x", x=64))

            # ---- main projections first (banded inputs ready earliest) ----
            # q projection kc-outer: all 6 oc accumulators live in PSUM at
            # once (3 x 2-bank tiles), so the PE starts when the first
            # (wq kc-strip, hsb kc-strip) DMA pair lands and streams while
            # the remaining strips arrive.
            qps = [psQ.tile([128, 1024], F32, tag="psQ", name=f"qps{i}")
                   for i in range(3)]
            for kc in range(KC):
                for oc in range(KC):
                    nc.tensor.matmul(
                        out=qps[oc // 2][:, 512 * (oc % 2):512 * (oc % 2) + 512],
                        lhsT=wq_t[:, kc, oc * 128:(oc + 1) * 128],
                        rhs=hsb[:, kc, C:C + 512],
                        start=(kc == 0),
                        stop=(kc == KC - 1 and not with_bias),
                    )
            for oc in range(KC):
                if with_bias:
                    nc.tensor.matmul(
                        out=qps[oc // 2][:, 512 * (oc % 2):512 * (oc % 2) + 512],
                        lhsT=bsb[1:2, oc * 128:(oc + 1) * 128],
                        rhs=bsb[0:1, 0:512], start=False, stop=True)
                nc.vector.tensor_copy(
                    out=qT[:, oc, :],
                    in_=qps[oc // 2][:, 512 * (oc % 2):512 * (oc % 2) + 512])
            proj_T(kT,
                   lambda kc, oc: wk_t[:, kc, oc * 128:(oc + 1) * 128],
                   ((256, 512), (0, 256), (768, 320)), 1, 0)
            w = load_w("wv")
            proj_nat(vE, w, (0, 1, 2, 3, 4, 5, 6, 7, 8), 2)

            # ---- global-row projections + partials (overlap banded) ----
            w = load_w("wkg")
            proj_T(kgT, lambda kc, oc, _w=w: _w[:, kc, oc * 128:(oc + 1) * 128],
                   ((C, 512),), 3, C)
            w = load_w("wvg")
            proj_nat(vgN, w, (2, 3, 4, 5), 4)
            w = load_w("wqg")
            # qgT [o, t] directly (N=64 matmuls; no PE transposes needed)
            proj_T(qgT, lambda kc, oc, _w=w: _w[:, kc, oc * 128:(oc + 1) * 128],
                   ((EXT, G),), 5, EXT)

            partial = dram.tile([G, NH * VW], F32)
            reduced = dram.tile([G, NH * VW], F32)

            def glob_pair(g0):
                # two global-row heads (g0 even, g0+1) share one PSUM tile and
                # one exp; scores land transposed ([keys, 64]) so PV needs no
                # transposes. Score matmuls alternate PE row tiles and pack.
                g1 = g0 + 1
                pse = psQ.tile([128, 1024], F32, tag="psQ", name=f"pse{g0}")
                for kt in range(4):
                    for gi, gh in enumerate((g0, g1)):
                        dd = 64 * (gh % 2)
                        nc.tensor.matmul(
                            out=pse[:, 512 * gi + kt * G:512 * gi + (kt + 1) * G],
                            lhsT=kgT[dd:dd + 64, gh // 2,
                                     128 * kt:128 * (kt + 1)],
                            rhs=qgT[dd:dd + 64, gh // 2, :],
                            start=True, stop=True)
                exgT = work.tile([128, 512], BF16, tag="exgT", name=f"exgT{g0}")
                for gi in range(2):
                    nc.scalar.activation(out=exgT[:, 256 * gi:256 * (gi + 1)],
                                         in_=pse[:, 512 * gi:512 * gi + 256],
                                         func=Exp, scale=SCALE)
                for gi, gh in enumerate((g0, g1)):
                    ppv = psO.tile([VW, TPC], F32, tag="psO", name=f"ppv{gh}")
                    for kt in range(4):
                        nc.tensor.matmul(
                            out=ppv[0:G, 0:VW],
                            lhsT=exgT[:, 256 * gi + kt * G:256 * gi + (kt + 1) * G],
                            rhs=vgN[:, kt, VW * gh:VW * (gh + 1)],
                            start=(kt == 0), stop=(kt == 3))
                    part = w2.tile([G, VW], F32, tag="part", name=f"part{gh}")
                    nc.vector.tensor_copy(out=part, in_=ppv[0:G, 0:VW])
                    nc.sync.dma_start(out=partial[:, gh * VW:(gh + 1) * VW],
                                      in_=part)

            # ---- banded + global-column attention (the bulk) ----
            # heads processed in pairs on disjoint PE row tiles with score
            # matmuls interleaved per key tile so the PE packs them; the
            # global-row pairs dovetail into the first banded pairs so the
            # AllReduce fires early and hides under the remaining banded work
            def emit_scores(hp):
                h0, h1 = 2 * hp, 2 * hp + 1
                pc = hp
                # -- scores for both heads, row tiles interleaved per jx --
                exs = {h0: [], h1: []}
                for pi, (pa, pb) in enumerate(JX_PAIRS):
                    wa, wb = JXW[pa], JXW[pb]
                    pss = {}
                    for hh in (h0, h1):
                        pss[hh] = psQ.tile([128, 1024], F32, tag="psQ",
                                           name=f"pss{hh}_{pa}")
                        ex = work.tile([128, 1024], BF16, tag="ex", bufs=18)
                        exs[hh].append(ex)
                    for jx, off in ((pa, 0), (pb, wa)):
                        t0 = JXT0[jx]
                        tn = t0 + JXW[jx]
                        for hh in (h0, h1):
                            dd = 64 * (hh % 2)
                            nc.tensor.matmul(
                                out=pss[hh][:, off:off + tn - t0],
                                lhsT=kT[dd:dd + 64, pc,
                                        128 * jx:128 * (jx + 1)],
                                rhs=qT[dd:dd + 64, pc, t0:tn],
                                start=True, stop=True)
                    # exp then mask per pair so the masks drain on vector
                    # while later pairs' scores still stream on the PE
                    _, (c0, c1), (k0, k1) = MASK_OPS[pi]
                    for hh in (h0, h1):
                        nc.scalar.activation(out=exs[hh][-1][:, 0:wa + wb],
                                             in_=pss[hh][:, 0:wa + wb],
                                             func=Exp, scale=SCALE)
                        nc.vector.tensor_mul(
                            exs[hh][-1][:, c0:c1], exs[hh][-1][:, c0:c1],
                            msb[:, k0:k1])
                # global-key columns for both heads; h1 lands in PSUM
                # partitions 64:128 of bank 2 (quadrant (64,64)) so its exp
                # stays lane-aligned and the K=64 PV tails pack as T0/T8
                pssg = psQ.tile([128, 1024], F32, tag="psQ", name=f"pssg{h0}")
                exg = work.tile([128, TPC], BF16, tag="exg", name=f"exg{h0}",
                                bufs=3)
                nc.tensor.matmul(
                    out=pssg[0:G, 0:TPC],
                    lhsT=kT[0:64, pc, EXT:EXT + G],
                    rhs=qT[0:64, pc, :],
                    start=True, stop=True)
                nc.tensor.matmul(
                    out=pssg[64:64 + G, 512:512 + TPC],
                    lhsT=kT[64:128, pc, EXT:EXT + G],
                    rhs=qT[64:128, pc, :],
                    start=True, stop=True)
                nc.scalar.activation(out=exg[0:G, :], in_=pssg[0:G, 0:TPC],
                                     func=Exp, scale=SCALE)
                nc.scalar.activation(out=exg[64:64 + G, :],
                                     in_=pssg[64:64 + G, 512:512 + TPC],
                                     func=Exp, scale=SCALE)
                return exs, exg

            def emit_pv(hp, exs, exg):
                h0, h1 = 2 * hp, 2 * hp + 1
                # -- PV per head; K=64 global tails adjacent so they pack --
                pso = {}
                for hh in (h0, h1):
                    pso[hh] = psO.tile([VW, TPC], F32, tag="psO",
                                       name=f"pso{hh}")
                    first_pv = True
                    for (pa, pb), ex in zip(JX_PAIRS, exs[hh]):
                        for jx, off in ((pa, 0), (pb, JXW[pa])):
                            t0 = JXT0[jx]
                            nc.tensor.matmul(
                                out=pso[hh][:, t0:t0 + JXW[jx]],
                                lhsT=vE[:, jx, VW * hh:VW * (hh + 1)],
                                rhs=ex[:, off:off + JXW[jx]],
                                start=first_pv, stop=False)
                            first_pv = False
                for hh, dd in ((h0, 0), (h1, 64)):
                    nc.tensor.matmul(
                        out=pso[hh],
                        lhsT=vE[dd:dd + G, 8, VW * hh:VW * (hh + 1)],
                        rhs=exg[dd:dd + G, :], start=False, stop=True)
                # raw [66,512] out block (values + denominator row); host
                # divides and transposes
                for i, hh in enumerate((h0, h1)):
                    ot = w2.tile([VW, TPC], BF16, tag="ot")
                    # split the two PSUM-evacuation casts across engines so
                    # the psO banks recycle ~0.6us earlier
                    if i == 0:
                        nc.vector.tensor_copy(out=ot, in_=pso[hh])
                    else:
                        nc.scalar.copy(out=ot, in_=pso[hh])
                    QS[(hp * 2 + i) % 2].dma_start(
                        out=d_out[VW * hh:VW * (hh + 1), :], in_=ot)

            # software pipeline: scores(hp) run while PV(hp-1) streams, so
            # the 64-row score bursts stay contiguous and pack on row tiles
            prev = None
            for hp in range(NH // 2):
                if hp == 0:
                    # all six pairs up front: every core's partials finish
                    # ~6us earlier, so the AllReduce (paced by the slowest
                    # core) starts and ends earlier in skew-bound runs
                    for g0 in range(0, NH, 2):
                        glob_pair(g0)
                if hp == 2:
                    nc.gpsimd.collective_compute(
                        "AllReduce", mybir.AluOpType.add,
                        replica_groups=[list(range(NCORE))],
                        ins=[partial.opt()], outs=[reduced.opt()])
                    red = late.tile([G, NH * VW], F32, tag="red")
                    nc.gpsimd.dma_start(out=red, in_=reduced)
                cur = emit_scores(hp)
                if prev is not None:
                    emit_pv(hp - 1, *prev)
                prev = cur
            emit_pv(NH // 2 - 1, *prev)

            # ---- ship the AllReduced global-row partials raw; the host
            # divides by the denominator columns (same as the banded blocks).
            # Only this one tail DMA depends on the collective, so a slow
            # peer core cannot stall any engine queue mid-kernel.
            nc.sync.dma_start(out=d_outg[:], in_=red)

    nc.compile()
    return nc


def _host_inputs(hs, weights, biases):
    """Build the 8 per-core input maps from full inputs."""
    BF = ml_dtypes.bfloat16
    hsT = np.ascontiguousarray(hs.T).astype(BF)    # [H, S]
    weights_bf = [w.astype(BF) for w in weights]

    with_bias = any(np.any(b) for b in biases)
    if with_bias:
        brow = np.zeros((7, COLS), BF)
        brow[0, :] = 1.0
        for i, b in enumerate(biases):
            brow[1 + i, :H] = b.astype(BF)
    pp = np.arange(128)[:, None]                    # key pos within jx tile
    in_maps = []
    for core in range(NCORE):
        hst = np.zeros((H, COLS), BF)
        lo = TPC * core - C
        hi = TPC * core + TPC + C
        clo, chi = max(lo, 0), min(hi, S)
        hst[:, clo - lo:chi - lo] = hsT[:, clo:chi]
        hst[:, EXT:EXT + G] = hsT[:, :G]
        hst[:, EXT + G:] = hsT[:, :G]               # duplicate global tokens
        # masks packed per pair: for packed col -> (jx, query q); valid iff
        # in-band and the absolute key is a real non-global token
        mk = np.ones((128, MCOLS), BF)
        for pi, (c0, c1), (k0, k1) in MASK_OPS:
            pa, pb = JX_PAIRS[pi]
            wa = JXW[pa]
            for ci in range(c0, c1):
                jx = pa if ci < wa else pb
                off = ci if ci < wa else ci - wa
                q = JXT0[jx] + off
                k_loc = 128 * jx - 256 + pp[:, 0]
                ka = TPC * core + k_loc
                valid = (np.abs(k_loc - q) <= C) & (ka >= G) & (ka < S)
                mk[:, k0 + (ci - c0)] = valid
        im = {
            "hsT": hst,
            "wq": weights_bf[0], "wk": weights_bf[1], "wv": weights_bf[2],
            "wkg": weights_bf[3], "wvg": weights_bf[4], "wqg": weights_bf[5],
            "masks": mk,
        }
        if with_bias:
            im["biasrow"] = brow
        in_maps.append(im)
    return in_maps, with_bias


def kernel(hidden_states, Wq, bq, Wk, bk, Wv, bv, Wqg, bqg, Wkg, bkg,
           Wvg, bvg):
    hs = np.asarray(hidden_states, np.float32).reshape(S, H)
    weights = [np.ascontiguousarray(np.asarray(w, np.float32))
               for w in (Wq, Wk, Wv, Wkg, Wvg, Wqg)]
    biases = [np.asarray(b, np.float32)
              for b in (bq, bk, bv, bkg, bvg, bqg)]
    in_maps, with_bias = _host_inputs(hs, weights, biases)

    if with_bias not in _PROG_CACHE:
        _PROG_CACHE[with_bias] = _build_program(with_bias)
    nc = _PROG_CACHE[with_bias]

    res = run_bass_kernel_spmd(nc, in_maps, list(range(NCORE)))

    out = np.empty((S, H), np.float32)
    for core in range(NCORE):
        blk = np.asarray(res.results[core]["out"], np.float32)
        blk = blk.reshape(NH, VW, TPC)
        norm = blk[:, :HD, :] / blk[:, HD:HD + 1, :]     # [NH, HD, TPC]
        out[TPC * core:TPC * (core + 1)] = (
            norm.transpose(2, 0, 1).reshape(TPC, H))
    og = np.asarray(res.results[0]["outg"], np.float32).reshape(G, NH, VW)
    out[:G] = (og[:, :, :HD] / og[:, :, HD:HD + 1]).reshape(G, H)
    return out.reshape(1, S, H)
